# revision 1
# baseline (speedup 1.0000x reference)
import numpy as np

HIDDEN = 4096
INTER = 14336
TOKENS = 4096
N_CORES = 8


def _kernel_jax(x, w_gate_up, w_down):
    import jax
    import jax.numpy as jnp
    from jax.sharding import Mesh, PartitionSpec as P, NamedSharding

    devs = jax.devices()
    if len(devs) < N_CORES:
        raise RuntimeError("need 8 devices")
    mesh = Mesh(np.asarray(devs[:N_CORES]), ("tp",))

    I = w_gate_up.shape[0] // 2
    w_gate = w_gate_up[:I]
    w_up = w_gate_up[I:]

    def f(x, wg, wu, wd):
        # column-parallel gate/up: each rank holds INTER/8 rows of each
        g = jnp.einsum("th,oh->to", x, wg)
        u = jnp.einsum("th,oh->to", x, wu)
        h = jax.nn.silu(g) * u
        # row-parallel down: contraction over the sharded axis -> all-reduce
        return jnp.einsum("ti,hi->th", h, wd)

    s = lambda spec: NamedSharding(mesh, spec)
    fj = jax.jit(
        f,
        in_shardings=(s(P(None, None)), s(P("tp", None)), s(P("tp", None)), s(P(None, "tp"))),
        out_shardings=s(P(None, None)),
    )
    out = fj(
        jnp.asarray(x, jnp.float32),
        jnp.asarray(w_gate, jnp.float32),
        jnp.asarray(w_up, jnp.float32),
        jnp.asarray(w_down, jnp.float32),
    )
    return np.asarray(jax.device_get(out), dtype=np.float32)


def _kernel_numpy(x, w_gate_up, w_down):
    I = w_gate_up.shape[0] // 2
    g = x @ w_gate_up[:I].T
    u = x @ w_gate_up[I:].T
    h = (g * (1.0 / (1.0 + np.exp(-g)))) * u
    return (h @ w_down.T).astype(np.float32)


def kernel(x, w_gate_up, w_down):
    x = np.asarray(x, dtype=np.float32)
    w_gate_up = np.asarray(w_gate_up, dtype=np.float32)
    w_down = np.asarray(w_down, dtype=np.float32)
    try:
        return _kernel_jax(x, w_gate_up, w_down)
    except Exception:
        return _kernel_numpy(x, w_gate_up, w_down)

